# revision 35
# baseline (speedup 1.0000x reference)
"""Multi-head attention forward on 8 Trainium2 NeuronCores.

Problem: batch=8, seq=1024, d_model=1024, n_heads=16, d_head=64, fp32 ref.

Sharding: data-parallel over batch - core b computes batch element b end to
end (weights replicated, no collectives).

Software-pipelined head-pair schedule: one long loop over head PAIRS in
which the PE rarely idles (keeps the HAM clock gate at K=8/8 = 2.4 GHz; a
phase-split kernel spends ~200us re-throttled at 1.2 GHz because the PE
sits idle behind softmax exp). Deliberately NOT packed to 100% PE
utilization: sustained all-engine saturation trips the chip's P0 power
state, downclocking every engine by 1.2x and losing more than the packing
gains (measured).

  prologue: Q/K projection pair 0 + V heads 0-7 s-tiles 0-4
  slot g:   scores(g) row-tile-paired + exp(g) + PV(g), interleaved with
            QK-proj(g+1) and the remaining V projections
  tail:     output projection, first two s-tiles' pair-0..6 partials
            emitted early to cover pair 7's normalize latency

Per-engine layout tricks:
  - scores^T = K @ Q^T per head has K(contraction)=64: the two heads of a
    pair live on partition halves 0-63 / 64-127, so their score matmuls
    auto-derive PE row-tile positions (0,0)/(64,0) and run CONCURRENTLY in
    the systolic array (2x score throughput).
  - softmax: exp on ScalarE with the 1/8 scale and key-mask folded in as
    activation scale/bias; denominators pop out of the PV matmul via a
    ones-column appended to V (psum row 64).
  - denominator reciprocals run on [128,8] tiles (two per pair) instead of
    [1,512] strips (which cost 4us each on 1 DVE lane).
  - b_Q/b_K are added by DVE tensor_scalar (per-partition scalar AP) during
    the psum->sbuf copy; b_V is folded into b_O on the host
    (out = (Z/den + b_V) W_O + b_O = Z/den W_O + (b_V W_O + b_O)).

Everything is bf16 into the PE with fp32 PSUM accumulation.

This toolchain's walrus encodes at most ONE sync wait per instruction;
_split_multi_waits hoists excess waits onto same-engine EventSemaphore
instructions.
"""

from contextlib import ExitStack

import numpy as np

import concourse.bass as bass
import concourse.tile as tile
from concourse import mybir
from concourse.bass_utils import run_bass_kernel_spmd

S = 1024  # seq
D = 1024  # d_model
H = 16  # heads
E = 64  # d_head
B = 8  # batch == n_cores
P = 128  # partitions
NS = S // P  # 8 s-tiles
ND = D // P  # 8 d-chunks
NG = H // 2  # 8 head pairs

F32 = mybir.dt.float32
BF16 = mybir.dt.bfloat16
AF = mybir.ActivationFunctionType
ADD = mybir.AluOpType.add

MASK_NEG = 60.0  # exp(x - 60) ~ 9e-27: masked keys vanish without inf/nan


def build_program(split_waits=True):
    nc = bass.Bass("TRN2", target_bir_lowering=False, debug=False)

    xt_d = nc.dram_tensor("xt", [P, ND, S], BF16, kind="ExternalInput").ap()
    wq_d = nc.dram_tensor("wq", [P, NG, ND, P], BF16, kind="ExternalInput").ap()
    wk_d = nc.dram_tensor("wk", [P, NG, ND, P], BF16, kind="ExternalInput").ap()
    wv_d = nc.dram_tensor("wv", [P, ND, H * E], BF16, kind="ExternalInput").ap()
    wo_d = nc.dram_tensor("wo", [P, NG, D], BF16, kind="ExternalInput").ap()
    # b_Q / b_K as per-pair partition columns: [:, 0:8]=b_Q, [:, 8:16]=b_K
    bqk_d = nc.dram_tensor("bqk", [P, 2 * NG], F32, kind="ExternalInput").ap()
    bo_d = nc.dram_tensor("bo", [1, D], BF16, kind="ExternalInput").ap()
    mb_d = nc.dram_tensor("mb", [P, NS], F32, kind="ExternalInput").ap()
    # bf16 output: halves the tail's output-DMA drain; host upcasts
    out_d = nc.dram_tensor("out", [S, D], BF16, kind="ExternalOutput").ap()

    with tile.TileContext(nc) as tc, ExitStack() as ctx:
        g1 = ctx.enter_context(tc.tile_pool(name="g1", bufs=1))

        ones_col = g1.tile([1, P], BF16, tag="ones_col")
        nc.vector.memset(ones_col, 1.0)

        # weights / activations, streamed in fine slices ordered so the
        # prologue's first matmuls start as early as possible
        wq_sb = g1.tile([P, NG, ND, P], BF16, tag="wq_sb")
        wk_sb = g1.tile([P, NG, ND, P], BF16, tag="wk_sb")
        xT = g1.tile([P, ND, S], BF16, tag="xT")
        wv_sb = g1.tile([P, ND, H * E], BF16, tag="wv_sb")
        wo_sb = g1.tile([P, NG, D], BF16, tag="wo_sb")
        mb_sb = g1.tile([P, NS], F32, tag="mb")
        bqk = g1.tile([P, 2 * NG], F32, tag="bqk")
        bo_sb = g1.tile([1, D], BF16, tag="bo")
        bo_bc = g1.tile([P, D], BF16, tag="bo_bc")
        # two HW DGE queues: Sync carries the qh0 halves + half the weights,
        # ScalarE's queue the qh1 halves + the other half (ScalarE is idle
        # until the first exp at ~25us and these issues clear before that).
        # Fine slicing keeps per-transfer latency ~1-2us so the c-outer
        # prologue can start consuming chunk 0 at ~11us instead of ~15us.
        # startup: sync carries qh0 halves, ScalarE's queue qh1 halves (its
        # 14 issues clear before the first exp; more would block the ScalarE
        # FIFO on DGE ring-full waits - measured +11us on the first exp).
        # Weight slices interleave with the x chunks they are consumed with.
        nc.sync.dma_start(out=bo_sb, in_=bo_d)  # dummy-warmer rhs: first
        nc.sync.dma_start(out=wq_sb[:, 0, 0:2], in_=wq_d[:, 0, 0:2])
        nc.scalar.dma_start(out=wk_sb[:, 0, 0:2], in_=wk_d[:, 0, 0:2])
        nc.sync.dma_start(out=bqk, in_=bqk_d)
        nc.sync.dma_start(out=mb_sb, in_=mb_d)
        for c in range(ND):
            if c % 2 == 0 and c > 0:
                nc.sync.dma_start(out=wq_sb[:, 0, c : c + 2],
                                  in_=wq_d[:, 0, c : c + 2])
                nc.scalar.dma_start(out=wk_sb[:, 0, c : c + 2],
                                    in_=wk_d[:, 0, c : c + 2])
            nc.sync.dma_start(out=xT[:, c, 0:512], in_=xt_d[:, c, 0:512])
            nc.scalar.dma_start(out=xT[:, c, 512:1024], in_=xt_d[:, c, 512:1024])
        for c in range(ND):
            # wv must fully land by ~26us (slot 0's first PV): even chunks
            # ride sync, odd chunks append to scalar's (short) startup list
            (nc.sync if c % 2 == 0 else nc.scalar).dma_start(
                out=wv_sb[:, c], in_=wv_d[:, c]
            )
        for g in range(1, NG):
            nc.sync.dma_start(out=wq_sb[:, g], in_=wq_d[:, g])
            nc.sync.dma_start(out=wk_sb[:, g], in_=wk_d[:, g])
        nc.sync.dma_start(out=wo_sb, in_=wo_d)
        nc.sync.dma_start(
            out=bo_bc, in_=bo_d.unsqueeze(1).broadcast_to((1, P, D))
        )

        # persistent activations
        qT = g1.tile([P, NG, S], BF16, tag="qT")
        kT = g1.tile([P, NG, S], BF16, tag="kT")
        vb = g1.tile([P, NS, H, E], BF16, tag="vb")
        zT = g1.tile([P, NG, S], BF16, tag="zT")
        ones_k = g1.tile([P, 1], BF16, tag="ones_k")
        nc.vector.memset(ones_k, 1.0)  # den matmul stationary

        # observer instructions: absorb one new semaphore tick each so later
        # consumers of shared tensors carry at most one wait themselves.
        nc.tensor.ldweights(ones_col)  # DVE tick (memsets)
        nc.tensor.ldweights(xT[:, 0, 0:8])  # xT chunk-0 DMA lane
        act_scrap = g1.tile([P, 1], F32, tag="act_scrap")
        nc.scalar.activation(  # mb DMA lane, observed by ScalarE
            out=act_scrap, in_=mb_sb[:, 0:1], func=AF.Copy
        )

        def qk_group(qp, dst, g, qh, w_sb, bcol):
            for c in range(ND):
                nc.tensor.matmul(
                    out=qp,
                    lhsT=w_sb[:, g, c, :],
                    rhs=xT[:, c, qh * 512 : (qh + 1) * 512],
                    start=(c == 0),
                    stop=(c == ND - 1),
                )
            with nc.allow_low_precision(reason="bf16 q/k with fused bias"):
                nc.vector.tensor_scalar(
                    out=dst[:, g, qh * 512 : (qh + 1) * 512],
                    in0=qp,
                    scalar1=bcol,
                    scalar2=None,
                    op0=ADD,
                )

        def v_group(vp, st, hh):
            for c in range(ND):
                nc.tensor.matmul(
                    out=vp,
                    lhsT=xT[:, c, st * P : (st + 1) * P],
                    rhs=wv_sb[:, c, hh * 512 : (hh + 1) * 512],
                    start=(c == 0),
                    stop=(c == ND - 1),
                )
            nc.vector.tensor_copy(
                out=vb[:, st, hh * 8 : (hh + 1) * 8, :],
                in_=vp.rearrange("p (h e) -> p h e", h=8),
            )

        # one pool scope for prologue + slots + tail: no mid-kernel pool
        # barriers, so the PE never drains at a phase edge
        with (
            tc.tile_pool(name="qpp", bufs=1, space="PSUM") as qpp,
            tc.tile_pool(name="stp", bufs=2, space="PSUM") as stp,
            tc.tile_pool(name="ztp", bufs=3, space="PSUM") as ztp,
            tc.tile_pool(name="ptp", bufs=16) as ptp,
            tc.tile_pool(name="zsbp", bufs=2) as zsbp,
            tc.tile_pool(name="denp", bufs=2) as denp,
            tc.tile_pool(name="rcp", bufs=2) as rcp,
            tc.tile_pool(name="bcp", bufs=2) as bcp,
            tc.tile_pool(name="obp", bufs=2) as obp,
            tc.tile_pool(name="stgp", bufs=6) as stgp,
        ):
            # prologue: pair-0 Q/K projections, chunk-outer so each x chunk
            # is consumed 4x as soon as its two DMA halves land (the DMA
            # supply rate paces the first ~10us). All V projections ride
            # slot worklists (emitted right before the PV that consumes
            # each s-tile) so nothing waits on the later wv DMA.
            def pe_warm():
                # zero-input matmul: keeps the PE HAM-warm through the
                # DMA-starved first ~25us (idle >3.4us re-throttles the
                # clock to 1.2 GHz for everything that follows)
                qp = qpp.tile([P, 512], F32, tag="qp", name="warm")
                nc.tensor.matmul(out=qp, lhsT=ones_col, rhs=bo_sb[:, 0:512],
                                 start=True, stop=True)

            qpro = stp.tile([P, S], F32, tag="st", name="qpro")
            kpro = stp.tile([P, S], F32, tag="st", name="kpro")
            for c in range(ND):
                for qh in range(2):
                    nc.tensor.matmul(
                        out=qpro[:, qh * 512 : (qh + 1) * 512],
                        lhsT=wq_sb[:, 0, c, :],
                        rhs=xT[:, c, qh * 512 : (qh + 1) * 512],
                        start=(c == 0),
                        stop=(c == ND - 1),
                    )
                    nc.tensor.matmul(
                        out=kpro[:, qh * 512 : (qh + 1) * 512],
                        lhsT=wk_sb[:, 0, c, :],
                        rhs=xT[:, c, qh * 512 : (qh + 1) * 512],
                        start=(c == 0),
                        stop=(c == ND - 1),
                    )
                if c < ND - 1:
                    pe_warm()
            with nc.allow_low_precision(reason="bf16 q/k with fused bias"):
                for qh in range(2):
                    nc.vector.tensor_scalar(
                        out=qT[:, 0, qh * 512 : (qh + 1) * 512],
                        in0=qpro[:, qh * 512 : (qh + 1) * 512],
                        scalar1=bqk[:, 0:1], scalar2=None, op0=ADD,
                    )
                    nc.vector.tensor_scalar(
                        out=kT[:, 0, qh * 512 : (qh + 1) * 512],
                        in0=kpro[:, qh * 512 : (qh + 1) * 512],
                        scalar1=bqk[:, NG : NG + 1], scalar2=None, op0=ADD,
                    )

            def finish_norm(g, zsb, denps):
                # dens sit in psum rows 0/32/64/96 of denps ((head, qh) =
                # (A,0),(A,1),(B,0),(B,1)). The psum->sbuf move rides the
                # (just-idled) ScalarE; one partition-step DMA spreads the
                # four rows over 128 partitions so the reciprocal isn't a
                # 1-lane (~4us) op; one DMA returns it to row layout; one
                # two-row broadcast DMA per qh builds the [128,512] scale.
                den_sb = denp.tile([P, 512], BF16, tag="densb", bufs=1,
                                   name=f"densb{g}")
                nc.scalar.activation(out=den_sb, in_=denps, func=AF.Copy)
                den2 = denp.tile([P, 16], BF16, tag="den", name=f"den2{g}")
                nc.sync.dma_start(out=den2, in_=den_sb[0 : 3 * 32 + 1 : 32, :])
                rc = rcp.tile([P, 16], BF16, tag="rc", name=f"rc{g}")
                with nc.allow_low_precision(reason="bf16 softmax denom"):
                    nc.vector.reciprocal(out=rc, in_=den2)
                rcs = rcp.tile([4, 512], BF16, tag="rcs", bufs=1,
                               name=f"rcs{g}")
                nc.sync.dma_start(out=rcs, in_=rc)
                for qh in range(2):
                    bc = bcp.tile([P, 512], BF16, tag="bc", name=f"bc{g}{qh}")
                    nc.sync.dma_start(
                        out=bc,
                        in_=rcs[qh : qh + 3 : 2, :].unsqueeze(1)
                        .broadcast_to((2, E, 512)),
                    )
                    # gpsimd (idle otherwise) keeps the bc-DMA wait out of
                    # the DVE FIFO; last pair on DVE - tail latency matters
                    eng = nc.vector if g == NG - 1 else nc.gpsimd
                    eng.tensor_mul(
                        zT[:, g, qh * 512 : (qh + 1) * 512],
                        zsb[:, qh, :],
                        bc,
                    )

            # staged output-projection partials: leading pairs of (st, dh),
            # accumulated through the qpp psum bank during slot 7's exp
            # stalls, +b_O folded in by the DVE move to bf16 SBUF staging.
            # The tail adds only the remaining pair(s), merging both dh
            # halves of an s-tile with one DVE add + one output DMA.
            stg = {}

            def o_unit(st, dh, n_pairs=NG - 1):
                op = qpp.tile([P, 512], F32, tag="qp", name=f"op{st}{dh}")
                for g2 in range(n_pairs):
                    nc.tensor.matmul(
                        out=op,
                        lhsT=zT[:, g2, st * P : (st + 1) * P],
                        rhs=wo_sb[:, g2, dh * 512 : (dh + 1) * 512],
                        start=(g2 == 0),
                        stop=(g2 == n_pairs - 1),
                    )
                if dh == 0:
                    stg[st] = (
                        stgp.tile([P, 2, 512], BF16, tag="stg",
                                  name=f"stg{st}"),
                        n_pairs,
                    )
                s, np0 = stg[st]
                assert np0 == n_pairs
                with nc.allow_low_precision(reason="bf16 o-proj partial"):
                    nc.vector.tensor_add(
                        s[:, dh, :], op, bo_bc[:, dh * 512 : (dh + 1) * 512]
                    )

            for g in range(NG):
                hA, hB = 2 * g, 2 * g + 1
                # filler matmul groups to keep the PE busy while ScalarE exps
                work = []
                vwork = [("v", st, 0) for st in range(NS)] if g == 0 else []
                vwork.reverse()
                if g < NG - 1:
                    for qh in range(2):
                        work.append(("qk", qT, g + 1, qh, wq_sb,
                                     bqk[:, g + 1 : g + 2]))
                    for qh in range(2):
                        work.append(("qk", kT, g + 1, qh, wk_sb,
                                     bqk[:, NG + g + 1 : NG + g + 2]))
                if g < 3:
                    for st in range(3 * g, min(3 * g + 3, NS)):
                        work.append(("v", st, 1))
                if g == NG - 1:
                    # slot 7 has no projection filler left: feed it the
                    # output-projection partials instead. The first two
                    # units stop at pair 5 (pair 6's normalize chain is
                    # still draining its den DMA round trips when they run).
                    work.append(("dummy",))
                    for i, (st, dh) in enumerate(
                        (st, dh) for st in range(4) for dh in range(2)
                    ):
                        work.append(("o", st, dh, 6 if i < 2 else NG - 1))
                work.reverse()

                def emit_one():
                    item = work.pop()
                    if item[0] == "o":
                        o_unit(item[1], item[2], item[3])
                        return
                    qp = qpp.tile([P, 512], F32, tag="qp", name="qp")
                    if item[0] == "qk":
                        qk_group(qp, *item[1:])
                    elif item[0] == "v":
                        v_group(qp, *item[1:])
                    else:
                        for _ in range(3):
                            nc.tensor.matmul(out=qp, lhsT=ones_col,
                                             rhs=bo_bc[0:1, 0:512],
                                             start=True, stop=True)

                pts = []
                ztAB0 = ztAB1 = denps = None
                for kt in range(NS + 1):
                    if kt < NS:
                        if g == 0 and kt < 6:
                            pe_warm()  # slot 0 is still DMA-paced
                        stA = stp.tile([P, S], F32, tag="st", name=f"stA{g}{kt}")
                        stB = stp.tile([P, S], F32, tag="st", name=f"stB{g}{kt}")
                        for qh in range(2):
                            # pair heads sit on partition halves -> PE
                            # row-tiles (0,0)/(64,0), concurrent in the array
                            nc.tensor.matmul(
                                out=stA[:, qh * 512 : (qh + 1) * 512],
                                lhsT=kT[0:E, g, kt * P : (kt + 1) * P],
                                rhs=qT[0:E, g, qh * 512 : (qh + 1) * 512],
                                start=True,
                                stop=True,
                            )
                            nc.tensor.matmul(
                                out=stB[:, qh * 512 : (qh + 1) * 512],
                                lhsT=kT[E:P, g, kt * P : (kt + 1) * P],
                                rhs=qT[E:P, g, qh * 512 : (qh + 1) * 512],
                                start=True,
                                stop=True,
                            )
                        ptA = ptp.tile([P, S], BF16, tag="pt", name=f"ptA{g}{kt}")
                        ptB = ptp.tile([P, S], BF16, tag="pt", name=f"ptB{g}{kt}")
                        nc.scalar.activation(
                            out=ptA, in_=stA, func=AF.Exp,
                            bias=mb_sb[:, kt : kt + 1], scale=0.125,
                        )
                        nc.scalar.activation(
                            out=ptB, in_=stB, func=AF.Exp,
                            bias=mb_sb[:, kt : kt + 1], scale=0.125,
                        )
                        pts.append((ptA, ptB))
                        if vwork:  # V for s-tile kt (after the scores so
                            # slot 0's first scores don't wait on the wv DMA)
                            item = vwork.pop()
                            v_group(qpp.tile([P, 512], F32, tag="qp",
                                             name="vq"), *item[1:])
                    if kt > 0:
                        # PV trails the scores/exp stream by one k-tile so it
                        # never waits on a just-finished exp. Head A occupies
                        # PE column groups 0-1, head B groups 2-3: both run
                        # CONCURRENTLY (col tiling), and the four softmax
                        # denominators stream as a third concurrent quad of
                        # M=1 col-tiled matmuls against a ones column.
                        pv = kt - 1
                        if pv == 0:
                            ztAB0 = ztp.tile([P, 512], F32, tag="zt",
                                             name=f"ztAB0{g}")
                            ztAB1 = ztp.tile([P, 512], F32, tag="zt",
                                             name=f"ztAB1{g}")
                            denps = ztp.tile([P, 512], F32, tag="zt",
                                             name=f"denps{g}")
                            nc.vector.memset(denps, 0.0)
                        for qh, zt in ((0, ztAB0), (1, ztAB1)):
                            nc.tensor.matmul(
                                out=zt[0:E, :], lhsT=vb[:, pv, hA, :],
                                rhs=pts[pv][0][:, qh * 512 : (qh + 1) * 512],
                                start=(pv == 0), stop=(pv == NS - 1),
                            )
                            nc.tensor.matmul(
                                out=zt[E:P, :], lhsT=vb[:, pv, hB, :],
                                rhs=pts[pv][1][:, qh * 512 : (qh + 1) * 512],
                                start=(pv == 0), stop=(pv == NS - 1),
                            )
                        for j, (h2, qh) in enumerate(
                            ((0, 0), (0, 1), (1, 0), (1, 1))
                        ):
                            nc.tensor.matmul(
                                out=denps[32 * j : 32 * j + 1, :],
                                lhsT=ones_k,
                                rhs=pts[pv][h2][:, qh * 512 : (qh + 1) * 512],
                                start=(pv == 0), stop=(pv == NS - 1),
                                tile_position=(0, 32 * j),
                                skip_group_check=True,
                            )
                    if kt < NS and work:
                        emit_one()
                while work:
                    emit_one()

                # z goes psum->sbuf immediately so psum banks recycle
                # without waiting on the normalize chain
                zsb = zsbp.tile([P, 2, 512], BF16, tag="zsb", name=f"zsb{g}")
                with nc.allow_low_precision(reason="bf16 z"):
                    nc.vector.tensor_copy(out=zsb[:, 0, :], in_=ztAB0)
                    nc.vector.tensor_copy(out=zsb[:, 1, :], in_=ztAB1)
                finish_norm(g, zsb, denps)

            # extra staged units for st 6,7: PE work that covers pair 7's
            # normalize chain (DVE copy -> DMA -> recip -> DMA -> mul) drain
            for st in (6, 7):
                for dh in range(2):
                    o_unit(st, dh)

            # tail: st 4,5 full accumulations first (b_O seeded via a K=1
            # matmul; not gated on zT pair 7), their pair-7 finish + ACT
            # psum->sbuf copies on the idle ScalarE free the stp ring for
            # the per-st singles: remaining pair(s) into a [P,S] psum, ONE
            # DVE merge with the staged partial, ONE [128,1024] output DMA.
            ops45 = {}
            for st in (4, 5):
                t = stp.tile([P, S], F32, tag="st", name=f"ot{st}")
                ops45[st] = t
                for dh in range(2):
                    nc.tensor.matmul(
                        out=t[:, dh * 512 : (dh + 1) * 512], lhsT=ones_col,
                        rhs=bo_sb[:, dh * 512 : (dh + 1) * 512],
                        start=True, stop=False,
                    )
                for g2 in range(NG - 1):
                    for dh in range(2):
                        nc.tensor.matmul(
                            out=t[:, dh * 512 : (dh + 1) * 512],
                            lhsT=zT[:, g2, st * P : (st + 1) * P],
                            rhs=wo_sb[:, g2, dh * 512 : (dh + 1) * 512],
                            start=False,
                            stop=False,
                        )

            def emit_out(ob, st, q):
                q.dma_start(out=out_d[st * P : (st + 1) * P, :], in_=ob)

            qs = [nc.sync, nc.scalar]
            for i, st in enumerate((4, 5)):
                for dh in range(2):
                    nc.tensor.matmul(
                        out=ops45[st][:, dh * 512 : (dh + 1) * 512],
                        lhsT=zT[:, NG - 1, st * P : (st + 1) * P],
                        rhs=wo_sb[:, NG - 1, dh * 512 : (dh + 1) * 512],
                        start=False, stop=True,
                    )
                ob = obp.tile([P, S], BF16, tag="ob", name=f"ob{st}")
                nc.scalar.activation(out=ob, in_=ops45[st], func=AF.Copy)
                emit_out(ob, st, qs[i % 2])
            for i, st in enumerate((0, 1, 2, 3, 6, 7)):
                s, n_pairs = stg[st]
                sp = stp.tile([P, S], F32, tag="st", name=f"sg{st}")
                rem = range(n_pairs, NG)
                for j, g2 in enumerate(rem):
                    for dh in range(2):
                        nc.tensor.matmul(
                            out=sp[:, dh * 512 : (dh + 1) * 512],
                            lhsT=zT[:, g2, st * P : (st + 1) * P],
                            rhs=wo_sb[:, g2, dh * 512 : (dh + 1) * 512],
                            start=(j == 0), stop=(j == len(rem) - 1),
                        )
                ob = obp.tile([P, S], BF16, tag="ob", name=f"ob{st}")
                with nc.allow_low_precision(reason="bf16 output"):
                    nc.vector.tensor_add(
                        ob, sp, s.rearrange("p a b -> p (a b)")
                    )
                emit_out(ob, st, qs[i % 2])

    if split_waits:
        _split_multi_waits(nc)
    return nc


def _split_multi_waits(nc):
    """This walrus build encodes at most ONE sync wait per instruction.
    Tile emits more. Hoist excess waits onto same-engine EventSemaphore
    instructions inserted immediately before the offender - engines and
    DGE sequencers execute their streams in order, so this preserves
    semantics exactly."""
    n = 0
    for fn in nc.m.functions:
        for bb in fn.blocks:
            out = []
            for inst in bb.instructions:
                si = getattr(inst, "sync_info", None)
                waits = list(si.on_wait) if si is not None and si.on_wait else []
                if len(waits) > 1:
                    for w in waits[:-1]:
                        n += 1
                        out.append(
                            mybir.InstEventSemaphore(
                                name=f"evw-{n}",
                                engine=inst.engine,
                                sync_info=mybir.SyncInfo(
                                    on_wait=[w], on_update=[]
                                ),
                            )
                        )
                    si.on_wait = [waits[-1]]
                out.append(inst)
            bb.instructions[:] = out


_NC_CACHE = None


def _get_nc():
    global _NC_CACHE
    if _NC_CACHE is None:
        _NC_CACHE = build_program()
    return _NC_CACHE


def _make_in_maps(inputs):
    import ml_dtypes

    bf16 = ml_dtypes.bfloat16
    x = np.asarray(inputs["x"], np.float32)
    mask = np.asarray(inputs["key_attention_mask"])
    wq = np.asarray(inputs["W_Q"], np.float32).astype(bf16)
    wk = np.asarray(inputs["W_K"], np.float32).astype(bf16)
    wv = np.asarray(inputs["W_V"], np.float32).astype(bf16)
    wo = np.asarray(inputs["W_O"], np.float32).astype(bf16)

    def pack_qk(w):  # (H, D, E) -> [p, g, c, (h2 e)]
        return np.ascontiguousarray(
            w.reshape(NG, 2, ND, P, E).transpose(3, 0, 2, 1, 4).reshape(P, NG, ND, P)
        )

    def pack_bcol(b):  # (H, E) -> [(h2 e), g]
        return b.reshape(NG, 2, E).transpose(1, 2, 0).reshape(P, NG)

    bqk = np.concatenate(
        [
            pack_bcol(np.asarray(inputs["b_Q"], np.float32)),
            pack_bcol(np.asarray(inputs["b_K"], np.float32)),
        ],
        axis=1,
    )
    # fold b_V into b_O: out = (Z/den + b_V) W_O + b_O
    bo = np.asarray(inputs["b_O"], np.float64) + np.einsum(
        "he,hed->d",
        np.asarray(inputs["b_V"], np.float64),
        np.asarray(inputs["W_O"], np.float64),
    )
    shared = {
        "wq": pack_qk(wq),
        "wk": pack_qk(wk),
        # (H, D, E) -> [p, c, (h e)]
        "wv": np.ascontiguousarray(
            wv.reshape(H, ND, P, E).transpose(2, 1, 0, 3).reshape(P, ND, H * E)
        ),
        # (H, E, D) -> [(h2 e), g, d]
        "wo": np.ascontiguousarray(
            wo.reshape(NG, 2, E, D).transpose(1, 2, 0, 3).reshape(P, NG, D)
        ),
        "bqk": np.ascontiguousarray(bqk),
        "bo": bo.astype(np.float32).astype(bf16).reshape(1, D),
    }
    in_maps = []
    for b in range(B):
        m = dict(shared)
        xt = x[b].T.astype(bf16)  # (D, S) -> [p, c, s]
        m["xt"] = np.ascontiguousarray(
            xt.reshape(ND, P, S).transpose(1, 0, 2)
        )
        mb = ((mask[b] != 0).astype(np.float32) - 1.0) * MASK_NEG
        m["mb"] = np.ascontiguousarray(mb.reshape(NS, P).T)
        in_maps.append(m)
    return in_maps


def run(inputs, trace=False):
    nc = _get_nc()
    res = run_bass_kernel_spmd(nc, _make_in_maps(inputs), list(range(B)),
                               trace=trace)
    out = np.stack(
        [np.asarray(res.results[b]["out"]).astype(np.float32) for b in range(B)],
        axis=0,
    )
    return out, res


def kernel(**inputs) -> np.ndarray:
    out, _ = run(inputs, trace=False)
    return out

